# revision 13
# baseline (speedup 1.0000x reference)
"""ChemProp message-to-node + MLP kernel for 8 TRN2 NeuronCores.

Strategy (no collectives needed):
  - Host assigns nodes to cores by global degree rank, round-robin, so
    each core receives exactly the edges destined for its nodes and all
    cores see near-identical degree sequences (minimal padding). Edge
    features are pre-permuted into a "degree-slot" layout so the
    device-side segment-sum is pure contiguous streaming adds.
  - Edge features stream in fp8 (e4m3) quantized on the host with
    per-node error feedback: edge d of a node is quantized carrying the
    residual of edge d-1, so the device's exact f32 PSUM accumulation
    telescopes and the segment-sum error is ~one quantization step
    (~0.6% rel) instead of sqrt(deg) steps. This halves the dominant
    HBM stream vs bf16.
  - Node groups of <=512 (one PSUM window). Within a group, nodes are
    sorted by degree (desc). Slot d holds the d-th edge of every node
    with degree > d. Consecutive slots are PAIRED and their columns
    interleaved so one fp8 DoubleRow matmul (256-deep contraction, 0.5
    PE cycles/col) adds both slots at once; slot-width mismatches are
    appended as plain single-slot matmuls (no stream padding).
  - Layout is feature-major ([256, cols] split into 2x128 partitions) so
    the MLP runs without any transposes: hidden^T = W1^T @ cat^T etc.
    r streams in fp8 as well; W1/W2 in bf16; MLP accumulates in f32
    PSUM; out is written bf16 and cast to f32 on host.
"""

import numpy as np
import ml_dtypes

import concourse.bacc as bacc
import concourse.mybir as mybir
import concourse.tile as tile
from concourse.bass_utils import run_bass_kernel_spmd
from concourse.masks import make_identity

NC = 8          # cores
GRP = 448       # nodes per group (one PSUM window)
TAIL_CAPS = (310, 290, 274)   # replaces the last full group + remainder
CHUNK = 4096    # stream-chunk columns
STREAM_BUFS = 10
MSG_BUFS = 2
PSUM_MSG_BUFS = 2
HID_BUFS = 2
OUT_BATCH = 2   # groups per out store strip

BF16 = mybir.dt.bfloat16
F32 = mybir.dt.float32
FP8 = mybir.dt.float8e4
NP_BF16 = ml_dtypes.bfloat16
NP_FP8 = ml_dtypes.float8_e4m3


# ----------------------------------------------------------------- host side
def _quantize_feedback(h, dst, n_nodes):
    """fp8(e4m3) quantization of edge features with per-node error feedback.

    Edges of a node are quantized in slot order; the quantization residual
    of edge d is added to edge d+1 before quantizing, so sum(q) telescopes
    to sum(h) - last_residual (device accumulates in exact f32)."""
    deg = np.bincount(dst, minlength=n_nodes)
    order = np.argsort(dst, kind="stable")
    starts = np.zeros(n_nodes + 1, dtype=np.int64)
    np.cumsum(deg, out=starts[1:])
    hq = np.zeros(h.shape, dtype=NP_FP8)
    carry = None
    nodes = None
    for d in range(int(deg.max())):
        prev_nodes = nodes
        nodes = np.nonzero(deg > d)[0] if d else np.arange(n_nodes)[deg > 0]
        eids = order[starts[nodes] + d]
        v = h[eids]
        if d:
            if len(nodes) == len(prev_nodes):
                v = v + carry
            else:
                # nodes is a prefix-subset of prev_nodes (deg>d implies deg>d-1)
                keep = np.isin(prev_nodes, nodes, assume_unique=True)
                v = v + carry[keep]
        q = v.astype(NP_FP8)
        hq[eids] = q
        carry = v - q.astype(np.float32)
    return hq, deg, order, starts


def _preprocess(deg_flat, order, starts, n_nodes, n_edges):
    """Build per-core layouts: node assignment, paired degree-slot column
    layout, per-group segment lists, col->edge maps."""
    npc = n_nodes // NC
    caps = [GRP] * (npc // GRP)
    rem = npc % GRP
    if TAIL_CAPS and caps and sum(TAIL_CAPS) == GRP + rem:
        caps = caps[:-1] + list(TAIL_CAPS)
    elif rem:
        caps.append(rem)
    grp_lo = np.concatenate([[0], np.cumsum(caps)]).astype(np.int64)
    ngrp = len(caps)

    # Node -> (core, position): global degree rank, round-robin over cores,
    # then round-robin over groups within the core; within a group positions
    # are filled degree-desc (slot-prefix property).
    rank = np.argsort(-deg_flat, kind="stable")
    node_ids = np.zeros((NC, npc), dtype=np.int64)
    deg_sorted = np.zeros((NC, npc), dtype=np.int64)
    for c in range(NC):
        ids_q = rank[c::NC]
        fill = [0] * ngrp
        for q in range(npc):
            g = q % ngrp
            while fill[g] == caps[g]:
                g = (g + 1) % ngrp
            pos = int(grp_lo[g]) + fill[g]
            fill[g] += 1
            node_ids[c, pos] = ids_q[q]
        deg_sorted[c] = deg_flat[node_ids[c]]

    # Regularized slot widths K[g][d] = max over cores of #nodes with deg > d
    # (slot 0 forced full group width so every msg column is initialized).
    # Slots are paired (0,1),(2,3),...; the common prefix of a pair becomes
    # an interleaved DoubleRow segment, the width difference a single-slot
    # segment (appended after all pair segments of the group).
    K = [None] * ngrp
    segs = [None] * ngrp       # list of (kind, src_c0, dst_j0, n)
    grp_col_lo = [0] * (ngrp + 1)
    off = 0
    for g in range(ngrp):
        lo, hi = int(grp_lo[g]), int(grp_lo[g + 1])
        w = hi - lo
        degs = deg_sorted[:, lo:hi]
        dmax = max(int(degs.max()), 1)
        counts = (degs[:, :, None] > np.arange(dmax)[None, None, :]).sum(1)
        Kg = counts.max(0)
        Kg[0] = w
        K[g] = Kg.astype(np.int64)
        sg = []
        singles = []
        for d0 in range(0, dmax, 2):
            k0 = int(Kg[d0])
            k1 = int(Kg[d0 + 1]) if d0 + 1 < dmax else 0
            if k1:
                sg.append(("pair", off, 0, k1, d0))
                off += 2 * k1
            if k0 > k1:
                singles.append((k0 - k1, k1, d0))
        for (n, j0, d0) in singles:
            sg.append(("single", off, j0, n, d0))
            off += n
        if off % 2:
            off += 1               # keep group starts even for pair APs
        segs[g] = sg
        grp_col_lo[g + 1] = off
    cols = off

    # col -> edge id (n_edges = zero pad), per core
    col_edge = np.full((NC, cols), n_edges, dtype=np.int64)
    for c in range(NC):
        for g in range(ngrp):
            lo = int(grp_lo[g])
            degs_g = deg_sorted[c, lo:int(grp_lo[g + 1])]
            for (kind, c0, j0, n, d0) in segs[g]:
                if kind == "pair":
                    for i, d in enumerate((d0, d0 + 1)):
                        kcd = int((degs_g > d).sum())
                        kcd = min(kcd, n)
                        if kcd == 0:
                            continue
                        nodes = node_ids[c, lo:lo + kcd]
                        col_edge[c, c0 + i:c0 + 2 * kcd + i:2] = \
                            order[starts[nodes] + d]
                else:
                    kcd = int((degs_g > d0).sum())
                    kcd = min(max(kcd - j0, 0), n)
                    if kcd == 0:
                        continue
                    nodes = node_ids[c, lo + j0:lo + j0 + kcd]
                    col_edge[c, c0:c0 + kcd] = order[starts[nodes] + d0]

    return {
        "npc": npc, "ngrp": ngrp, "cols": cols,
        "segs": segs, "node_ids": node_ids,
        "col_edge": col_edge, "grp_lo": grp_lo,
    }


def _build_streams(hq, r, lay):
    """Materialize per-core device input arrays (fp8 streams)."""
    n_edges, Fdim = hq.shape
    npc, cols = lay["npc"], lay["cols"]
    fp = Fdim // 128

    h_aug = np.zeros((n_edges + 1, Fdim), dtype=NP_FP8)
    h_aug[:n_edges] = hq
    hs, rT = [], []
    for c in range(NC):
        block = h_aug[lay["col_edge"][c]]            # [cols, F]
        hs.append(np.ascontiguousarray(block.T).reshape(fp, 128, cols))
        rc = r[lay["node_ids"][c]].astype(NP_FP8)    # [npc, F]
        # pair-interleaved for DoubleRow rhs: rT[p, i, n] = r[n, i*128+p]
        rT.append(np.ascontiguousarray(
            rc.T.reshape(fp, 128, npc).transpose(1, 0, 2)))
    return hs, rT


# --------------------------------------------------------------- device side
def _build_graph(lay, Fdim, H, Fout):
    npc, ngrp, cols = lay["npc"], lay["ngrp"], lay["cols"]
    fp = Fdim // 128          # 2 feature ptiles
    kt_n = (2 * Fdim) // 128  # 4 k-chunks for W1
    ht_n = H // 128           # 4 hidden ptiles
    ot_n = Fout // 128        # 2 output ptiles

    nc = bacc.Bacc(None, target_bir_lowering=False)
    hs_p = nc.declare_dram_parameter("hs", [fp, 128, cols], FP8, isOutput=False)
    rT_p = nc.declare_dram_parameter("rT", [128, fp, npc], FP8, isOutput=False)
    # W1 top half (r rows) in fp8, pair-interleaved for DoubleRow:
    # w1r[p, ht*256 + i*128 + m] = W1[i*128 + p, ht*128 + m]
    w1r_p = nc.declare_dram_parameter("W1r8", [128, fp * H], FP8,
                                      isOutput=False)
    # W1 bottom half (msg rows) in bf16
    w1_p = nc.declare_dram_parameter("W1b", [fp, 128, H], BF16, isOutput=False)
    w2_p = nc.declare_dram_parameter("W2", [ht_n, 128, Fout], BF16,
                                     isOutput=False)
    out_p = nc.declare_dram_parameter("out", [ot_n, 128, npc], BF16,
                                      isOutput=True)

    n_chunks = (cols + CHUNK - 1) // CHUNK

    with tile.TileContext(nc) as tc:
        with (
            tc.tile_pool(name="const", bufs=1) as const_pool,
            tc.tile_pool(name="stream", bufs=STREAM_BUFS) as stream_pool,
            tc.tile_pool(name="msgp", bufs=PSUM_MSG_BUFS, space="PSUM") as msg_psum_pool,
            tc.tile_pool(name="msgb", bufs=MSG_BUFS) as msg_pool,
            tc.tile_pool(name="mlp1p", bufs=2, space="PSUM") as mlp1_psum_pool,
            tc.tile_pool(name="mlp2p", bufs=2, space="PSUM") as mlp2_psum_pool,
            tc.tile_pool(name="hid", bufs=HID_BUFS) as hid_pool,
            tc.tile_pool(name="osb", bufs=2) as out_pool,
        ):
            # [I | I] fp8 identity pair for DoubleRow adds -- built FIRST so
            # the PE isn't stalled behind the weight DMAs on the Pool engine
            ident2 = const_pool.tile([128, 256], FP8, tag="ident2")
            make_identity(nc, ident2[:, 0:128])
            make_identity(nc, ident2[:, 128:256])
            id_pair = ident2[:].rearrange("p (two m) -> p two m", two=2)
            id_one = ident2[:, 0:128]
            # weights + full rT resident in SBUF (rT is only 12.5KB/partition;
            # one big up-front DMA avoids per-group SWDGE churn that can jam
            # the descriptor ring and stall the PE)
            w1r_sb = const_pool.tile([128, fp * H], FP8, tag="w1r")
            nc.gpsimd.dma_start(out=w1r_sb[:], in_=w1r_p[:, :])
            w1_sb = []
            for k in range(fp):
                t = const_pool.tile([128, H], BF16, tag=f"w1_{k}")
                nc.gpsimd.dma_start(out=t[:], in_=w1_p[k])
                w1_sb.append(t)
            w2_sb = []
            for k in range(ht_n):
                t = const_pool.tile([128, Fout], BF16, tag=f"w2_{k}")
                nc.gpsimd.dma_start(out=t[:], in_=w2_p[k])
                w2_sb.append(t)
            rT_sb = const_pool.tile([128, fp, npc], FP8, tag="rT")
            nc.gpsimd.dma_start(out=rT_sb[:], in_=rT_p[:, :, :])

            chunk_tiles = [[None] * n_chunks for _ in range(fp)]

            def get_chunk(p, ci):
                if chunk_tiles[p][ci] is None:
                    w = min(CHUNK, cols - ci * CHUNK)
                    t = stream_pool.tile([128, w], FP8, tag=f"hs{p}")
                    nc.sync.dma_start(
                        out=t[:], in_=hs_p[p, :, ci * CHUNK:ci * CHUNK + w])
                    chunk_tiles[p][ci] = t
                return chunk_tiles[p][ci]

            for g in range(ngrp):
                lo = int(lay["grp_lo"][g])
                w_g = int(lay["grp_lo"][g + 1]) - lo

                # ---- split segments on chunk boundaries
                pieces = []   # (kind, chunk, src_off, dst_off, n_out)
                for (kind, c0, j0, n, _d0) in lay["segs"][g]:
                    span = 2 * n if kind == "pair" else n
                    s = c0
                    while s < c0 + span:
                        ci = s // CHUNK
                        e = min(c0 + span, (ci + 1) * CHUNK)
                        if kind == "pair":
                            pieces.append(("pair", ci, s - ci * CHUNK,
                                           j0 + (s - c0) // 2, (e - s) // 2))
                        else:
                            pieces.append(("single", ci, s - ci * CHUNK,
                                           j0 + (s - c0), e - s))
                        s = e

                # ---- segment-sum: fp8 identity matmuls accumulate in PSUM.
                # Exactly ONE start=True per PSUM window (a second one resets
                # the bank's has_written bits); untouched columns first-touch
                # via has_written=0 on their first start=False.
                msgb = []
                for p in range(fp):
                    ps = msg_psum_pool.tile([128, w_g], F32, space="PSUM",
                                            tag=f"mp{p}")
                    for i, (kind, ci, o0, dj, n) in enumerate(pieces):
                        src = get_chunk(p, ci)
                        if kind == "pair":
                            nc.tensor.matmul(
                                out=ps[:, dj:dj + n],
                                lhsT=id_pair,
                                rhs=src[:, o0:o0 + 2 * n].rearrange(
                                    "p (n two) -> p two n", two=2),
                                start=(i == 0), stop=(i == len(pieces) - 1),
                                perf_mode=mybir.MatmulPerfMode.DoubleRow,
                                skip_group_check=True)
                        else:
                            nc.tensor.matmul(
                                out=ps[:, dj:dj + n],
                                lhsT=id_one,
                                rhs=src[:, o0:o0 + n],
                                start=(i == 0), stop=(i == len(pieces) - 1),
                                skip_group_check=True)
                    mb = msg_pool.tile([128, w_g], BF16, tag=f"mb{p}")
                    nc.vector.tensor_copy(out=mb[:], in_=ps[:])
                    msgb.append(mb)

                # ---- r slice (fp8, pair-interleaved, resident in SBUF)
                rb = rT_sb[:, :, lo:lo + w_g]

                # ---- MLP: hidden^T = relu(W1^T @ cat^T); the r half runs as
                # one fp8 DoubleRow matmul (256-deep, 0.5 cyc/col), the msg
                # half as two bf16 matmuls accumulating into the same PSUM
                hid = []
                for ht in range(ht_n):
                    ps = mlp1_psum_pool.tile([128, w_g], F32, space="PSUM",
                                             tag="mlp1")
                    nc.tensor.matmul(
                        out=ps[:],
                        lhsT=w1r_sb[:, ht * 256:(ht + 1) * 256].rearrange(
                            "p (two m) -> p two m", two=2),
                        rhs=rb,
                        start=True, stop=False,
                        perf_mode=mybir.MatmulPerfMode.DoubleRow,
                        skip_group_check=True)
                    for k in range(fp):
                        nc.tensor.matmul(
                            out=ps[:],
                            lhsT=w1_sb[k][:, ht * 128:(ht + 1) * 128],
                            rhs=msgb[k][:],
                            start=False, stop=(k == fp - 1),
                            skip_group_check=True)
                    hb = hid_pool.tile([128, w_g], BF16, tag=f"h{ht}")
                    nc.scalar.activation(
                        hb[:], ps[:], mybir.ActivationFunctionType.Relu)
                    hid.append(hb)

                # ---- out^T = W2^T @ hidden^T
                for ot in range(ot_n):
                    ps = mlp2_psum_pool.tile([128, w_g], F32, space="PSUM",
                                             tag="mlp2")
                    for k in range(ht_n):
                        nc.tensor.matmul(
                            out=ps[:],
                            lhsT=w2_sb[k][:, ot * 128:(ot + 1) * 128],
                            rhs=hid[k][:],
                            start=(k == 0), stop=(k == ht_n - 1))
                    if g % OUT_BATCH == 0 and ot == 0:
                        ob_lo = lo
                        ob_hi = int(lay["grp_lo"][min(g + OUT_BATCH, ngrp)])
                        ob_strips = []
                        for o in range(ot_n):
                            ob_t = out_pool.tile([128, ob_hi - ob_lo],
                                                 BF16, tag=f"o{o}")
                            ob_strips.append(ob_t)
                    nc.scalar.activation(
                        ob_strips[ot][:, lo - ob_lo:lo - ob_lo + w_g],
                        ps[:], mybir.ActivationFunctionType.Copy)
                    if g % OUT_BATCH == OUT_BATCH - 1 or g == ngrp - 1:
                        nc.scalar.dma_start(
                            out=out_p[ot, :, ob_lo:ob_lo + ob_strips[ot].shape[1]],
                            in_=ob_strips[ot][:])

    nc.finalize()
    return nc


# ----------------------------------------------------------------- interface
def prepare(r, h, nbrs, W1, W2):
    """Preprocess inputs + build the Bass graph. Returns everything needed
    to run and to assemble the output."""
    r = np.asarray(r, dtype=np.float32)
    h = np.asarray(h, dtype=np.float32)
    nbrs = np.asarray(nbrs)
    W1 = np.asarray(W1, dtype=np.float32)
    W2 = np.asarray(W2, dtype=np.float32)

    n_nodes, Fdim = r.shape
    n_edges = h.shape[0]
    H = W1.shape[1]
    Fout = W2.shape[1]

    dst = nbrs[:, 0].astype(np.int64)
    hq, deg, order, starts = _quantize_feedback(h, dst, n_nodes)
    lay = _preprocess(deg, order, starts, n_nodes, n_edges)
    hs, rT = _build_streams(hq, r, lay)
    fp = Fdim // 128
    htn = H // 128
    # W1 top (r rows): fp8, pair-interleaved per hidden tile for DoubleRow
    w1r = np.ascontiguousarray(
        W1[:Fdim].astype(NP_FP8).reshape(fp, 128, htn, 128)
        .transpose(1, 2, 0, 3).reshape(128, fp * H))
    # W1 bottom (msg rows): bf16 k-tiles
    w1b = np.ascontiguousarray(W1[Fdim:].astype(NP_BF16)).reshape(fp, 128, H)
    w2d = np.ascontiguousarray(W2.astype(NP_BF16)).reshape(-1, 128, Fout)

    nc = _build_graph(lay, Fdim, H, Fout)
    in_maps = [
        {"hs": hs[c], "rT": rT[c], "W1r8": w1r, "W1b": w1b, "W2": w2d}
        for c in range(NC)
    ]
    return {"nc": nc, "in_maps": in_maps, "lay": lay,
            "n_nodes": n_nodes, "Fout": Fout}


def assemble(prep, results):
    lay = prep["lay"]
    n_nodes, Fout = prep["n_nodes"], prep["Fout"]
    npc = lay["npc"]
    out = np.zeros((n_nodes, Fout), dtype=np.float32)
    for c in range(NC):
        o = np.asarray(results[c]["out"]).reshape(Fout, npc)
        out[lay["node_ids"][c]] = o.T.astype(np.float32)
    return out


def kernel(r, h, nbrs, W1, W2):
    prep = prepare(r, h, nbrs, W1, W2)
    res = run_bass_kernel_spmd(prep["nc"], prep["in_maps"],
                               core_ids=list(range(NC)))
    return assemble(prep, res.results)
